# revision 3
# baseline (speedup 1.0000x reference)
"""Trainium2 Bass kernel for nn_Attentioncross (gnn_message_passing), v3.

Reference computation, per node n (N=50000) and row r (R=8), D=256:
    idx[r] = [r, r+1, r-1] (with idx[0]=[0,1,2], idx[7]=[7,6,5])
    s[n,j]   = W2 @ leaky_relu(W1 @ z[n,j,:], 0.01)        (scalar per row)
    beta     = softmax([s[self], s[j1], s[j2]])            (over the 3)
    o[n,r,:] = z[n,r,:] + beta1*z[n,j1,:] + beta2*z[n,j2,:]

Data-parallel over N across 8 cores; rows tile as [128, 256] (16
nodes/tile, neighbor structure intra-tile). Inputs: z natural f16
(combine + residual) and a host-pre-transposed f16 copy zt (d on
partitions, score path) so no on-chip transposes are needed.

Scores: ht = leaky(W1 zt) over 512-row quads; per-tile s = ht^T W2 puts
per-row scores on partitions; softmax over {self, j1, j2} on [128, G]
vectors; tiny PE shift-gather matmuls produce partition-shifted beta
columns c1/c2/c3.

Combine per tile: o = C^T z, one PE matmul against
C = P1 + Y where P1 = M1A*c1 (ScalarE activation with per-partition
scale) and Y = I + MEX*c3 + M2A*c2 (two chained DVE
scalar_tensor_tensor ops); the final add P1 + Y runs on gpsimd
(SBUF-only tensor_tensor — real HW gpsimd supports neither
scalar_tensor_tensor nor PSUM access). The shifted-beta columns are
staged from PSUM into SBUF (f32) so all scalar operands are
gpsimd/walrus-legal. PSUM evacuation of o is batched 4 tiles per
instruction (3 batches ScalarE, 1 DVE); output is f16, cast to f32 on
host.

The issue order software-pipelines supertiles: loads(st+2), C builds
(st), score phase(st+1), combine(st) — so PE's in-order stream runs the
next supertile's matmuls while the C pieces are built, and output
drains (Act queue) never block input prefetch (SP queue).
"""
import sys

for p in ("/opt/trn_rl_repo",):
    if p not in sys.path:
        sys.path.insert(0, p)

import numpy as np
from contextlib import ExitStack

N_FULL, R, D = 50000, 8, 256
N_CORES = 8
NODES_PER_TILE = 16          # 128 rows / 8
P = 128
TILES_PER_CORE = 391         # 6256 nodes/core * 8 rows / 128
NODES_PER_CORE = TILES_PER_CORE * NODES_PER_TILE   # 6256
N_PAD = NODES_PER_CORE * N_CORES                   # 50048
ROWS_PER_CORE = NODES_PER_CORE * R                 # 50048
SUPER = 16                   # tiles per supertile
QROWS = 512                  # rows per W1-matmul quad

# static 3-neighbor pattern (matches reference._neighbor_idx for R=8)
J1 = [1, 2, 3, 4, 5, 6, 7, 6]
J2 = [2, 0, 1, 2, 3, 4, 5, 5]

# mask indices in the packed const tensor [128, NMASK, 128]
M_G1, M_G2, M_SH1, M_SH2, M_A1, M_A2, M_1A, M_2A, M_EX, M_I = range(10)
NMASK = 10


def _build_masks():
    # C[j, r] = I + b1[r]*[j==J1[r]] + b2[r]*[j==J2[r]] per 8-block, split as
    #   X = M1A * c1          (tensor_scalar,  scalar per partition j)
    #   Y = (MEX * c3 + I) then + M2A * c2    (chained stt)
    # with partition-shifted beta columns
    #   c1[j] = b1[j-1]            (regular J1: j = r+1, r in 0..6)
    #   c2[j] = b2[j+1]            (regular J2: j = r-1, r in 1..6)
    #   c3[6] = b1[7], c3[2] = b2[0], c3[5] = b2[7]    (exceptions)
    m = np.zeros((P, NMASK, P), np.float16)
    for b in range(NODES_PER_TILE):
        g = b * R
        for r in range(R):
            # score gathers: s1[r] = sum_j g1[j, r] s[j]
            m[g + J1[r], M_G1, g + r] = 1.0
            m[g + J2[r], M_G2, g + r] = 1.0
            m[g + r, M_I, g + r] = 1.0
        # beta partition-shift gathers (lhsT[r, j]; c[j] = sum_r lhsT[r,j] b[r])
        for r in range(7):
            m[g + r, M_SH1, g + r + 1] = 1.0          # c1[r+1] = b1[r]
        for r in range(1, 7):
            m[g + r, M_SH2, g + r - 1] = 1.0          # c2[r-1] = b2[r]
        m[g + 7, M_A1, g + 6] = 1.0                   # c3[6] = b1[7]
        m[g + 0, M_A2, g + 2] = 1.0                   # c3[2] = b2[0]
        m[g + 7, M_A2, g + 5] = 1.0                   # c3[5] = b2[7]
        # C-side masks (mask[j, r])
        for r in range(7):
            m[g + r + 1, M_1A, g + r] = 1.0           # j = r+1
        for r in range(1, 7):
            m[g + r - 1, M_2A, g + r] = 1.0           # j = r-1
        m[g + 6, M_EX, g + 7] = 1.0
        m[g + 2, M_EX, g + 0] = 1.0
        m[g + 5, M_EX, g + 7] = 1.0
    return m


def _build_consts(W1, W2):
    masks = _build_masks()
    w1t = np.ascontiguousarray(
        W1.T.reshape(2, P, 16).transpose(1, 0, 2)
    ).astype(np.float16)  # [128, 2, 16]
    w2c = np.ascontiguousarray(W2.reshape(16, 1)).astype(np.float16)  # [16, 1]
    return masks, w1t, w2c


def _build_nc():
    import concourse.bacc as bacc
    import concourse.tile as tile
    from concourse import mybir

    f32 = mybir.dt.float32
    f16 = mybir.dt.float16

    nc = bacc.Bacc("TRN2", target_bir_lowering=False)
    z_d = nc.declare_dram_parameter("z", [ROWS_PER_CORE, D], f16, isOutput=False)
    zt_d = nc.declare_dram_parameter("zt", [2, P, ROWS_PER_CORE], f16, isOutput=False)
    masks_d = nc.declare_dram_parameter("masks", [P, NMASK, P], f16, isOutput=False)
    w1_d = nc.declare_dram_parameter("w1t", [P, 2, 16], f16, isOutput=False)
    w2_d = nc.declare_dram_parameter("w2c", [16, 1], f16, isOutput=False)
    o_d = nc.declare_dram_parameter("o", [ROWS_PER_CORE, D], f16, isOutput=True)

    Prelu = mybir.ActivationFunctionType.Prelu
    Exp = mybir.ActivationFunctionType.Exp
    Copy = mybir.ActivationFunctionType.Copy
    add = mybir.AluOpType.add
    mult = mybir.AluOpType.mult

    n_super = (TILES_PER_CORE + SUPER - 1) // SUPER

    with tile.TileContext(nc) as tc, ExitStack() as ctx:
        consts = ctx.enter_context(tc.tile_pool(name="consts", bufs=1))
        zpool = ctx.enter_context(tc.tile_pool(name="zp", bufs=3))
        ztpool = ctx.enter_context(tc.tile_pool(name="ztp", bufs=3))
        opool = ctx.enter_context(tc.tile_pool(name="op", bufs=2))
        htpool = ctx.enter_context(tc.tile_pool(name="htp", bufs=2))
        small = ctx.enter_context(tc.tile_pool(name="small", bufs=2))
        cpool = ctx.enter_context(tc.tile_pool(name="cp", bufs=2))

        ps_ht = ctx.enter_context(tc.tile_pool(name="ps_ht", bufs=1, space="PSUM"))
        ps_sc = ctx.enter_context(tc.tile_pool(name="ps_sc", bufs=2, space="PSUM"))
        ps_o = ctx.enter_context(tc.tile_pool(name="ps_o", bufs=2, space="PSUM"))

        masks_sb = consts.tile([P, NMASK, P], f16)
        w1_sb = consts.tile([P, 2, 16], f16)
        w2_sb = consts.tile([16, 1], f16)
        nc.sync.dma_start(out=masks_sb, in_=masks_d[:])
        nc.sync.dma_start(out=w1_sb, in_=w1_d[:])
        nc.sync.dma_start(out=w2_sb, in_=w2_d[:])
        g1_m = masks_sb[:, M_G1, :]
        g2_m = masks_sb[:, M_G2, :]
        sh1_m = masks_sb[:, M_SH1, :]
        sh2_m = masks_sb[:, M_SH2, :]
        a1_m = masks_sb[:, M_A1, :]
        a2_m = masks_sb[:, M_A2, :]
        m1a_m = masks_sb[:, M_1A, :]
        m2a_m = masks_sb[:, M_2A, :]
        mex_m = masks_sb[:, M_EX, :]
        i_m = masks_sb[:, M_I, :]

        state = {}

        def issue_load(st):
            g0 = st * SUPER
            G = min(SUPER, TILES_PER_CORE - g0)
            row0, nrow = g0 * P, G * P
            z_sb = zpool.tile([P, SUPER, D], f16, tag="z", name="z_sb")
            zt_sb = ztpool.tile([P, 2, SUPER * P], f16, tag="zt", name="zt_sb")
            # chunk the first two supertiles' loads so the prologue's first
            # W1 matmuls start after a quarter-load instead of a full one
            nch = 4 if st < 2 else 1
            step = (nrow + nch - 1) // nch
            for k in range(0, nrow, step):
                m = min(step, nrow - k)
                nc.sync.dma_start(
                    out=zt_sb[:, :, k : k + m],
                    in_=zt_d[:, :, row0 + k : row0 + k + m].rearrange(
                        "c p r -> p c r"
                    ),
                )
            for k in range(0, nrow, step):
                m = min(step, nrow - k)
                nc.sync.dma_start(
                    out=z_sb[:, k // P : (k + m) // P, :],
                    in_=z_d[row0 + k : row0 + k + m, :].rearrange(
                        "(g p) d -> p g d", p=P
                    ),
                )
            state[st] = {"G": G, "row0": row0, "nrow": nrow, "z": z_sb, "zt": zt_sb}

        def issue_scores_a(st):
            S = state[st]
            G, nrow, zt_sb = S["G"], S["nrow"], S["zt"]
            ht_sb = htpool.tile([16, SUPER * P], f16, tag="ht", name="ht_sb")
            nq = (nrow + QROWS - 1) // QROWS
            for q0 in range(0, nq, 2):
                npair = min(2 * QROWS, nrow - q0 * QROWS)
                ht_ps = ps_ht.tile([16, 2, QROWS], f32, tag="ht", name="ht_ps")
                for q in (0, 1):
                    m = min(QROWS, npair - q * QROWS)
                    if m <= 0:
                        break
                    for c in range(2):
                        nc.tensor.matmul(
                            ht_ps[:, q, 0:m],
                            w1_sb[:, c, :],
                            zt_sb[:, c, (q0 + q) * QROWS : (q0 + q) * QROWS + m],
                            start=(c == 0),
                            stop=(c == 1),
                        )
                nc.scalar.activation(
                    ht_sb[:, q0 * QROWS : q0 * QROWS + npair],
                    ht_ps.rearrange("h q r -> h (q r)")[:, 0:npair],
                    Prelu,
                    alpha=0.01,
                )
            # psum bank layout: 0:S s | S:3S s12 | 3S:6S c1/c2/c3
            sc = ps_sc.tile([P, 6 * SUPER], f32, tag="sc", name="sc")
            for t in range(G):
                nc.tensor.matmul(
                    sc[:, t : t + 1],
                    ht_sb[:, t * P : (t + 1) * P],
                    w2_sb,
                    start=True,
                    stop=True,
                )
            S["sc"] = sc

        def issue_scores_b(st):
            S = state[st]
            G, sc = S["G"], S["sc"]
            s_sb = small.tile([P, SUPER], f16, tag="ssb", name="s_sb")
            nc.vector.tensor_copy(s_sb[:, 0:G], sc[:, 0:G])
            nc.tensor.matmul(
                sc[:, SUPER : SUPER + G], g1_m, s_sb[:, 0:G], start=True, stop=True
            )
            nc.tensor.matmul(
                sc[:, 2 * SUPER : 2 * SUPER + G], g2_m, s_sb[:, 0:G],
                start=True, stop=True,
            )
            e0 = small.tile([P, SUPER], f32, tag="e0", name="e0")
            e12 = small.tile([P, 2, SUPER], f32, tag="e12", name="e12")
            nc.scalar.activation(e0[:, 0:G], sc[:, 0:G], Exp)
            nc.scalar.activation(
                e12[:, :, 0:G],
                sc[:, SUPER : 3 * SUPER].rearrange("p (k g) -> p k g", k=2)[:, :, 0:G],
                Exp,
            )
            den = small.tile([P, SUPER], f32, tag="den", name="den")
            nc.vector.tensor_tensor(den[:, 0:G], e0[:, 0:G], e12[:, 0, 0:G], add)
            nc.vector.tensor_tensor(den[:, 0:G], den[:, 0:G], e12[:, 1, 0:G], add)
            rden = small.tile([P, SUPER], f32, tag="rden", name="rden")
            nc.vector.reciprocal(rden[:, 0:G], den[:, 0:G])
            b12 = small.tile([P, 2, SUPER], f16, tag="b12", name="b12")
            nc.vector.tensor_tensor(b12[:, 0, 0:G], e12[:, 0, 0:G], rden[:, 0:G], mult)
            nc.vector.tensor_tensor(b12[:, 1, 0:G], e12[:, 1, 0:G], rden[:, 0:G], mult)
            # partition-shifted beta columns c1/c2/c3 (psum cols 3S:6S)
            nc.tensor.matmul(
                sc[:, 3 * SUPER : 3 * SUPER + G], sh1_m, b12[:, 0, 0:G],
                start=True, stop=True,
            )
            nc.tensor.matmul(
                sc[:, 4 * SUPER : 4 * SUPER + G], sh2_m, b12[:, 1, 0:G],
                start=True, stop=True,
            )
            nc.tensor.matmul(
                sc[:, 5 * SUPER : 5 * SUPER + G], a1_m, b12[:, 0, 0:G],
                start=True, stop=False,
            )
            nc.tensor.matmul(
                sc[:, 5 * SUPER : 5 * SUPER + G], a2_m, b12[:, 1, 0:G],
                start=False, stop=True,
            )
            # gpsimd cannot read PSUM on HW: stage the shifted-beta columns
            # into SBUF for the C-build scalar operands
            csh = small.tile([P, 3 * SUPER], f32, tag="csh", name="csh")
            nc.vector.tensor_copy(csh, sc[:, 3 * SUPER : 6 * SUPER])
            S["csh"] = csh

        def issue_cbuilds(st, lo, hi):
            # gpsimd supports neither scalar_tensor_tensor nor PSUM access on
            # real HW: Act builds the M1A*c1 piece (activation scale), DVE
            # chains I + MEX*c3 + M2A*c2, gpsimd adds the two (SBUF only)
            S = state[st]
            G, csh = S["G"], S["csh"]
            cs = S.setdefault("cs", {})
            for t in range(lo, min(hi, G)):
                p1 = cpool.tile([P, P], f16, tag=f"p_{t}", name=f"p{t}")
                nc.scalar.activation(
                    p1, m1a_m, Copy, scale=csh[:, t : t + 1]
                )
                ct = cpool.tile([P, P], f16, tag=f"t_{t}", name=f"t{t}")
                nc.vector.scalar_tensor_tensor(
                    ct, mex_m, csh[:, 2 * SUPER + t : 2 * SUPER + t + 1], i_m,
                    op0=mult, op1=add,
                )
                y = cpool.tile([P, P], f16, tag=f"y_{t}", name=f"y{t}")
                nc.vector.scalar_tensor_tensor(
                    y, m2a_m, csh[:, SUPER + t : SUPER + t + 1], ct,
                    op0=mult, op1=add,
                )
                c = cpool.tile([P, P], f16, tag=f"c_{t}", name=f"c{t}")
                nc.gpsimd.tensor_tensor(c, p1, y, add)
                cs[t] = c

        def issue_combines(st, lo, hi):
            S = state[st]
            G, z_sb, cs = S["G"], S["z"], S["cs"]
            if lo == 0:
                S["o_sb"] = opool.tile([P, SUPER, D], f16, tag="o", name="o_sb")
            o_sb = S["o_sb"]
            o_ps = S.get("o_ps")
            for t in range(lo, min(hi, G)):
                if t % 4 == 0:
                    o_ps = ps_o.tile([P, 4, D], f32, tag="ops", name="o_ps")
                nc.tensor.matmul(
                    o_ps[:, t % 4, :], cs[t], z_sb[:, t, :], start=True, stop=True
                )
                if t % 4 == 3 or t == G - 1:
                    w = (t % 4) + 1
                    b0 = t - w + 1
                    if t // 4 == 3:
                        nc.vector.tensor_copy(
                            o_sb[:, b0 : b0 + w, :], o_ps[:, 0:w, :]
                        )
                    else:
                        nc.scalar.activation(
                            o_sb[:, b0 : b0 + w, :], o_ps[:, 0:w, :], Copy
                        )
            S["o_ps"] = o_ps
            if hi >= G:
                row0, nrow = S["row0"], S["nrow"]
                nc.scalar.dma_start(
                    out=o_d[row0 : row0 + nrow, :].rearrange("(g p) d -> p g d", p=P),
                    in_=o_sb[:, 0:G, :],
                )
                del state[st]

        # software-pipelined schedule; loads prefetched 2 supertiles ahead;
        # combine halves interleaved with the next supertile's score phase
        issue_load(0)
        if n_super > 1:
            issue_load(1)
        issue_scores_a(0)
        issue_scores_b(0)
        for st in range(n_super):
            if st + 2 < n_super:
                issue_load(st + 2)
            issue_cbuilds(st, 0, SUPER)
            if st + 1 < n_super:
                issue_scores_a(st + 1)
                issue_scores_b(st + 1)
            issue_combines(st, 0, SUPER)

    nc.finalize()
    return nc


_NC_CACHE = None


def _get_nc():
    global _NC_CACHE
    if _NC_CACHE is None:
        _NC_CACHE = _build_nc()
    return _NC_CACHE


def _prepare_in_maps(z, W1, W2):
    z = np.asarray(z, dtype=np.float32)
    zp = np.zeros((N_PAD, R, D), np.float32)
    zp[: z.shape[0]] = z

    masks, w1t, w2c = _build_consts(
        np.asarray(W1, np.float32), np.asarray(W2, np.float32)
    )
    in_maps = []
    for c in range(N_CORES):
        sl = slice(c * NODES_PER_CORE, (c + 1) * NODES_PER_CORE)
        zc = zp[sl].reshape(ROWS_PER_CORE, D)
        in_maps.append(
            {
                "z": np.ascontiguousarray(zc).astype(np.float16),
                "zt": np.ascontiguousarray(
                    zc.reshape(ROWS_PER_CORE, 2, P).transpose(1, 2, 0)
                ).astype(np.float16),
                "masks": masks,
                "w1t": w1t,
                "w2c": w2c,
            }
        )
    return in_maps


def _gather_out(res, n):
    out = np.empty((N_PAD, R, D), np.float32)
    for c in range(N_CORES):
        out[c * NODES_PER_CORE : (c + 1) * NODES_PER_CORE] = (
            res.results[c]["o"].astype(np.float32).reshape(NODES_PER_CORE, R, D)
        )
    return out[:n]


def kernel(z, W1, W2):
    from concourse.bass_utils import run_bass_kernel_spmd

    nc = _get_nc()
    in_maps = _prepare_in_maps(z, W1, W2)
    res = run_bass_kernel_spmd(nc, in_maps, core_ids=list(range(N_CORES)))
    return _gather_out(res, np.asarray(z).shape[0])
